# revision 1
# baseline (speedup 1.0000x reference)
"""Mueller-matrix pyramid kernel for Trainium2 (8 NeuronCores).

Sharding: 8 cores = (batch 4) x (H-halves 2). Each core computes the full
51-channel output for its 256-row half at 512 cols.

Per-core layout: channel-planes with pixels on [partitions=rows, free=cols].
- Mueller features: DVE tensor_tensor chain (adjugate/cofactor form; the
  det(A)*det(W) scale cancels in the m00 normalization), ACT reciprocal.
- Pyramid pooling: W-pool = strided DVE max; H-pool = partition-strided
  SBUF->SBUF DMA gathers + DVE max.
- Levels 1/2 features share one DVE pass on packed [68, 136] planes
  (lvl1 cols 0:128, lvl2 packed into cols 128:136; 68*8 == 17*32).
- Bilinear upsample (align_corners): PE matmuls (transpose -> W-matmul with
  the data as stationary operand -> H-matmul with per-core R matrices).

SPMD uniformity: halo rows (16 extra pooling rows) arrive in a separate
input tensor; their pooled rows live at fixed local positions and the
per-core R1/R2 matrices absorb the local->global row permutation, so one
program serves both halves.
"""

import numpy as np

H = W = 512
CIN = 48
LEVELS = 3
HALF = 256          # output rows per core
HALO = 16           # extra pooling rows per core
L1R = 68            # local level-1 rows (64 main + 4 halo)
L1W = 128
L2R = 17            # local level-2 rows (16 main + 1 halo)
L2W = 32
PACK2 = 8           # lvl2 packed cols per partition-row (68*8 = 17*32)
FTW = L1W + PACK2   # 136
N_CORES = 8
CW = 256            # level-0 col-tile width

# ---------------------------------------------------------------------------
# host-side constants
# ---------------------------------------------------------------------------


def _interp_1d(n_out, n_in, lo, hi):
    out = np.zeros((hi - lo, n_in), np.float32)
    scale = (n_in - 1.0) / (n_out - 1.0)
    for i, y in enumerate(range(lo, hi)):
        t = np.float32(y * scale)
        y0 = int(np.floor(t))
        fy = np.float32(t - y0)
        y1 = min(y0 + 1, n_in - 1)
        out[i, y0] += np.float32(1.0) - fy
        out[i, y1] += fy
    return out


def _r_matrix(half, n_in, n_main, off_main, off_halo, n_halo):
    lo, hi = half * HALF, half * HALF + HALF
    full = _interp_1d(H, n_in, lo, hi)
    loc = np.zeros((HALF, n_main + n_halo), np.float32)
    loc[:, :n_main] = full[:, off_main:off_main + n_main]
    loc[:, n_main:] = full[:, off_halo:off_halo + n_halo]
    return loc


def _host_constants(half):
    if half == 0:
        r1 = _r_matrix(0, 128, 64, 0, 64, 4)
        r2 = _r_matrix(0, 32, 16, 0, 16, 1)
    else:
        r1 = _r_matrix(1, 128, 64, 64, 60, 4)
        r2 = _r_matrix(1, 32, 16, 16, 15, 1)
    c1 = _interp_1d(W, L1W, 0, W).T.copy()
    c2 = _interp_1d(W, L2W, 0, W).T.copy()
    return (np.ascontiguousarray(r1.T), np.ascontiguousarray(r2.T),
            np.ascontiguousarray(c1), np.ascontiguousarray(c2))


# ---------------------------------------------------------------------------
# op tables (adjugate via cofactors); a[k] = entry (k//4, k%4)
# ---------------------------------------------------------------------------

# minors m = a[e1]*a[e2] - a[e3]*a[e4]
# S-minors s0..s5 (rows 0,1), C-minors c0..c5 (rows 2,3)
_SMIN = [
    (0, 5, 4, 1), (0, 6, 4, 2), (0, 7, 4, 3),
    (1, 6, 5, 2), (1, 7, 5, 3), (2, 7, 6, 3),
]
_CMIN = [
    (8, 13, 12, 9), (8, 14, 12, 10), (8, 15, 12, 11),
    (9, 14, 13, 10), (9, 15, 13, 11), (10, 15, 14, 11),
]
# adj[flat] = sign*(a[x1]*m1 - a[x2]*m2 + a[x3]*m3); minors: ('c'|'s', idx)
_ADJ = {
    0:  (+1, (5, 'c', 5), (6, 'c', 4), (7, 'c', 3)),
    1:  (-1, (1, 'c', 5), (2, 'c', 4), (3, 'c', 3)),
    2:  (+1, (13, 's', 5), (14, 's', 4), (15, 's', 3)),
    3:  (-1, (9, 's', 5), (10, 's', 4), (11, 's', 3)),
    4:  (-1, (4, 'c', 5), (6, 'c', 2), (7, 'c', 1)),
    5:  (+1, (0, 'c', 5), (2, 'c', 2), (3, 'c', 1)),
    6:  (-1, (12, 's', 5), (14, 's', 2), (15, 's', 1)),
    7:  (+1, (8, 's', 5), (10, 's', 2), (11, 's', 1)),
    8:  (+1, (4, 'c', 4), (5, 'c', 2), (7, 'c', 0)),
    9:  (-1, (0, 'c', 4), (1, 'c', 2), (3, 'c', 0)),
    10: (+1, (12, 's', 4), (13, 's', 2), (15, 's', 0)),
    11: (-1, (8, 's', 4), (9, 's', 2), (11, 's', 0)),
    12: (-1, (4, 'c', 3), (5, 'c', 1), (6, 'c', 0)),
    13: (+1, (0, 'c', 3), (1, 'c', 1), (2, 'c', 0)),
    14: (-1, (12, 's', 3), (13, 's', 1), (14, 's', 0)),
    15: (+1, (8, 's', 3), (9, 's', 1), (10, 's', 0)),
}

_NC_CACHE = {}


def _build_nc(repeat=1):
    import concourse.bacc as bacc
    import concourse.mybir as mybir
    from concourse.tile import TileContext
    from concourse.masks import make_identity

    f32 = mybir.dt.float32
    ALU = mybir.AluOpType
    AF = mybir.ActivationFunctionType

    nc = bacc.Bacc("TRN2", target_bir_lowering=False, num_devices=N_CORES)

    xmm = nc.dram_tensor("xmm", [CIN, HALF, W], f32, kind="ExternalInput")
    xhalo = nc.dram_tensor("xhalo", [CIN, HALO, W], f32, kind="ExternalInput")
    r1t = nc.dram_tensor("r1t", [L1R, HALF], f32, kind="ExternalInput")
    r2t = nc.dram_tensor("r2t", [L2R, HALF], f32, kind="ExternalInput")
    c1 = nc.dram_tensor("c1", [L1W, W], f32, kind="ExternalInput")
    c2 = nc.dram_tensor("c2", [L2W, W], f32, kind="ExternalInput")
    out = nc.dram_tensor("out", [17 * LEVELS, HALF, W], f32, kind="ExternalOutput")
    import os
    dbg_en = os.environ.get("KDBG") == "1"
    dbg = (nc.dram_tensor("dbg", [L1R, CIN, FTW], f32, kind="ExternalOutput")
           if dbg_en else None)

    def TT(o, a, b, op, eng=None):
        (eng or nc.vector).tensor_tensor(out=o, in0=a, in1=b, op=op)

    # DVE fp32 TT ~= (FD+58)/0.96 ns; Pool TT ~= 2.6*FD/1.2 ns.
    # Interleave independent chains across both engines, weighted so each
    # finishes together. POOL_EVERY=3 -> ~1/3 of chains on Pool.
    POOL_EVERY = 3

    def mueller(pool_t, FD, xI, xA, xW, opl, rs, xIr=None, oplr=None,
                use_pool=True):
        """Emit the 48->17 Mueller feature chain on [rs, FD] planes.
        xI/xA/xW: accessor(e)->plane AP for matrix entry e (flat 0..15).
        opl(k): output plane (0 = intensity, 1+4i+j = M[i,j])."""
        mnr = pool_t.tile([128, 6, FD], f32, tag="mnr")
        adjc = pool_t.tile([128, 4, FD], f32, tag="adjc")
        pp = pool_t.tile([128, 16, FD], f32, tag="pp")
        scr = pool_t.tile([128, 7, FD], f32, tag="scr")

        def pl(t, k):
            return t[0:rs, k]

        s1 = pl(scr, 2)
        chain_ctr = [0]

        def pick():
            """Engine + scratch plane for the next independent chain."""
            chain_ctr[0] += 1
            if use_pool and chain_ctr[0] % POOL_EVERY == 0:
                return nc.gpsimd, pl(scr, 1)
            return nc.vector, pl(scr, 0)

        # intensity: pairwise tree using out planes 1:9 as scratch
        TT(oplr(1, 9), xIr(0, 8), xIr(8, 16), ALU.add)
        TT(oplr(1, 5), oplr(1, 5), oplr(5, 9), ALU.add)
        TT(oplr(1, 3), oplr(1, 3), oplr(3, 5), ALU.add)
        TT(s1, oplr(1, 2).squeeze(1), oplr(2, 3).squeeze(1), ALU.add)
        nc.scalar.mul(opl(0), s1, 1.0 / 16.0)

        def emit_minors(xE, table):
            for i, (e1, e2, e3, e4) in enumerate(table):
                eng, s0 = pick()
                TT(pl(mnr, i), xE(e1), xE(e2), ALU.mult, eng)
                TT(s0, xE(e3), xE(e4), ALU.mult, eng)
                TT(pl(mnr, i), pl(mnr, i), s0, ALU.subtract, eng)

        def emit_adj_entry(xE, dst, flat):
            eng, s0 = pick()
            sgn, t1, t2, t3 = _ADJ[flat]
            def mslot(t):
                return pl(mnr, t[2])
            if sgn > 0:
                TT(dst, xE(t1[0]), mslot(t1), ALU.mult, eng)
                TT(s0, xE(t2[0]), mslot(t2), ALU.mult, eng)
                TT(dst, dst, s0, ALU.subtract, eng)
                TT(s0, xE(t3[0]), mslot(t3), ALU.mult, eng)
                TT(dst, dst, s0, ALU.add, eng)
            else:
                TT(dst, xE(t2[0]), mslot(t2), ALU.mult, eng)
                TT(s0, xE(t1[0]), mslot(t1), ALU.mult, eng)
                TT(dst, dst, s0, ALU.subtract, eng)
                TT(s0, xE(t3[0]), mslot(t3), ALU.mult, eng)
                TT(dst, dst, s0, ALU.subtract, eng)

        # ---- P = adj(A) @ I, accumulated column-by-column of adjA ----
        # batched: pp[i,j] (+)= adjc[i] * I[4k+j] as one [rs,4,4,FD] op,
        # with out planes 1:17 as the k>0 product scratch
        ppflat = pp[0:rs, 0:16]
        pp4 = ppflat.rearrange("r (i j) w -> r i j w", j=4)
        for mtype, table, cols in (('c', _CMIN, (0, 1)), ('s', _SMIN, (2, 3))):
            emit_minors(xA, table)
            for k in cols:
                for i in range(4):
                    emit_adj_entry(xA, pl(adjc, i), 4 * i + k)
                a4 = adjc[0:rs, 0:4].unsqueeze(2).broadcast_to((rs, 4, 4, FD))
                i4 = xIr(4 * k, 4 * k + 4).unsqueeze(1).broadcast_to(
                    (rs, 4, 4, FD))
                if k == 0:
                    TT(pp4, a4, i4, ALU.mult)
                else:
                    prodflat = oplr(1, 17)
                    prod4 = prodflat.rearrange("r (i j) w -> r i j w", j=4)
                    TT(prod4, a4, i4, ALU.mult)
                    TT(ppflat, ppflat, prodflat, ALU.add)

        # ---- N = P @ adj(W): raw N accumulated into out planes 1:17,
        # normalization deferred to one batched multiply at the end ----
        pp4d = ppflat.rearrange("r (i k) w -> r i k w", k=4)
        otr = oplr(1, 17).rearrange("r (i j) w -> r i j w", j=4)
        prod4 = scr[0:rs, 3:7]
        for mtype, table, cols in (('c', _CMIN, (0, 1)), ('s', _SMIN, (2, 3))):
            emit_minors(xW, table)
            for j in cols:
                for k in range(4):
                    emit_adj_entry(xW, pl(adjc, k), 4 * k + j)
                otcol = otr[:, :, j]
                for k in range(4):
                    ak = adjc[0:rs, k].unsqueeze(1).broadcast_to((rs, 4, FD))
                    if k == 0:
                        TT(otcol, pp4d[:, :, 0], ak, ALU.mult)
                    else:
                        TT(prod4, pp4d[:, :, k], ak, ALU.mult)
                        TT(otcol, otcol, prod4, ALU.add)
        rec = s1
        nc.vector.reciprocal(rec, opl(1))
        r16 = rec.unsqueeze(1).broadcast_to((rs, 16, FD))
        TT(oplr(1, 17), oplr(1, 17), r16, ALU.mult)

    with TileContext(nc) as tc:
        with (
            tc.tile_pool(name="cst", bufs=1) as pool_c,
            tc.tile_pool(name="lvl", bufs=1) as pool_l,
        ):
            ident = pool_c.tile([128, 128], f32)
            make_identity(nc, ident)
            r1t_s = pool_c.tile([L1R, HALF], f32)
            nc.sync.dma_start(out=r1t_s, in_=r1t[:, :])
            r2t_s = pool_c.tile([L2R, HALF], f32)
            nc.sync.dma_start(out=r2t_s, in_=r2t[:, :])
            c1_s = pool_c.tile([L1W, W], f32)
            nc.sync.dma_start(out=c1_s, in_=c1[:, :])
            c2_s = pool_c.tile([L2W, W], f32)
            nc.sync.dma_start(out=c2_s, in_=c2[:, :])

            for _rep in range(repeat):
                # persistent level-1(+packed lvl2) planes
                lvl1 = pool_l.tile([L1R, CIN, FTW], f32)

                # ================= phase 0: level-0 tiles + pooling ============
                with (
                    tc.tile_pool(name="px", bufs=1) as pool_x,
                    tc.tile_pool(name="pt", bufs=1) as pool_t,
                    tc.tile_pool(name="po", bufs=1) as pool_o,
                    tc.tile_pool(name="pp", bufs=1) as pool_p,
                    tc.tile_pool(name="phl", bufs=1) as pool_h,
                ):
                    for rt in range(2):
                        for ct in range(2):
                            r0, c0 = rt * 128, ct * CW
                            chunks = []
                            for ci, tg in ((0, "xi"), (1, "xa"), (2, "xw")):
                                t = pool_x.tile([128, 16, CW], f32, tag=tg)
                                nc.sync.dma_start(
                                    out=t,
                                    in_=xmm[16 * ci:16 * ci + 16, r0:r0 + 128,
                                            c0:c0 + CW].rearrange("c r w -> r c w"))
                                chunks.append(t)
                            xi, xa, xw = chunks
                            ot = pool_o.tile([128, 17, CW], f32, tag="ot")
                            mueller(pool_t, CW,
                                    lambda e: xi[:, e], lambda e: xa[:, e],
                                    lambda e: xw[:, e], lambda k: ot[:, k], 128,
                                    xIr=lambda lo, hi: xi[:, lo:hi],
                                    oplr=lambda lo, hi: ot[:, lo:hi])
                            nc.sync.dma_start(
                                out=out[0:17, r0:r0 + 128, c0:c0 + CW]
                                .rearrange("c r w -> r c w"),
                                in_=ot)
                            # W-pool all 48 chans -> wp [128, 48, 64]
                            wp = pool_p.tile([128, CIN, CW // 4], f32, tag="wp")
                            for ci, t in enumerate(chunks):
                                t4 = t.rearrange("r c (w f) -> r c w f", f=4)
                                wps = wp[:, 16 * ci:16 * ci + 16]
                                TT(wps, t4[:, :, :, 0], t4[:, :, :, 1], ALU.max)
                                TT(wps, wps, t4[:, :, :, 2], ALU.max)
                                TT(wps, wps, t4[:, :, :, 3], ALU.max)
                            # H-pool -> lvl1[32rt:32rt+32, :, 64ct:64ct+64]
                            wpr = wp.rearrange("(r f) c w -> r f c w", f=4)
                            dst = lvl1[32 * rt:32 * rt + 32, :,
                                       64 * ct:64 * ct + 64]
                            nc.sync.dma_start(out=dst, in_=wpr[:, 0])
                            gt = pool_p.tile([128, CIN, CW // 4], f32, tag="gt")
                            gts = gt[32 * rt:32 * rt + 32, :, 0:64]
                            for dy in (1, 2, 3):
                                nc.sync.dma_start(out=gts, in_=wpr[:, dy])
                                TT(dst, dst, gts, ALU.max)

                    # halo strip, rows-on-partitions like the main path
                    HCW = 128
                    for ct in range(W // HCW):
                        c0 = ct * HCW
                        xh = pool_h.tile([HALO, CIN, HCW], f32, tag="xh")
                        nc.sync.dma_start(
                            out=xh,
                            in_=xhalo[:, :, c0:c0 + HCW].rearrange("c r w -> r c w"))
                        wph = pool_h.tile([HALO, CIN, HCW // 4], f32, tag="wph")
                        xh4 = xh.rearrange("r c (w f) -> r c w f", f=4)
                        TT(wph, xh4[:, :, :, 0], xh4[:, :, :, 1], ALU.max)
                        TT(wph, wph, xh4[:, :, :, 2], ALU.max)
                        TT(wph, wph, xh4[:, :, :, 3], ALU.max)
                        wphr = wph.rearrange("(r f) c w -> r f c w", f=4)
                        dst = lvl1[64:68, :, 32 * ct:32 * ct + 32]
                        nc.sync.dma_start(out=dst, in_=wphr[:, 0])
                        gh = pool_h.tile([68, CIN, HCW // 4], f32, tag="gh")
                        ghs = gh[64:68]
                        for dy in (1, 2, 3):
                            nc.sync.dma_start(out=ghs, in_=wphr[:, dy])
                            TT(dst, dst, ghs, ALU.max)

                # ================= phase 1: levels 1+2 ========================
                with (
                    tc.tile_pool(name="pt2", bufs=1) as pool_t2,
                    tc.tile_pool(name="pl2", bufs=1) as pool_2,
                    tc.tile_pool(name="po2", bufs=1) as pool_o2,
                    tc.tile_pool(name="pu", bufs=2) as pool_u,
                    tc.tile_pool(name="psu", bufs=2, space="PSUM") as pool_ps,
                ):
                    # --- level-2 build into lvl1 cols 128:136 ---
                    wp2 = pool_2.tile([L1R, CIN, L2W], f32, tag="wp2")
                    l14 = lvl1[:, :, 0:L1W].rearrange("r c (w f) -> r c w f", f=4)
                    TT(wp2, l14[:, :, :, 0], l14[:, :, :, 1], ALU.max)
                    TT(wp2, wp2, l14[:, :, :, 2], ALU.max)
                    TT(wp2, wp2, l14[:, :, :, 3], ALU.max)
                    wp2r = wp2[0:64].rearrange("(r f) c w -> r f c w", f=4)
                    l2 = pool_2.tile([L2R, CIN, L2W], f32, tag="l2")
                    l2m = l2[0:16]
                    nc.sync.dma_start(out=l2m, in_=wp2r[:, 0])
                    g2 = pool_2.tile([L2R, CIN, L2W], f32, tag="g2")
                    g2m = g2[0:16]
                    for dy in (1, 2, 3):
                        nc.sync.dma_start(out=g2m, in_=wp2r[:, dy])
                        TT(l2m, l2m, g2m, ALU.max)
                    th = pool_2.tile([1, CIN, L2W], f32, tag="th")
                    gh2 = pool_2.tile([1, CIN, L2W], f32, tag="gh2")
                    nc.sync.dma_start(out=th, in_=wp2[64:65])
                    for dy in (1, 2, 3):
                        nc.sync.dma_start(out=gh2, in_=wp2[64 + dy:65 + dy])
                        TT(th, th, gh2, ALU.max)
                    nc.sync.dma_start(out=l2[16:17], in_=th)
                    lvl1p = lvl1.rearrange("(r f) c w -> r f c w", f=4)
                    for m in range(4):
                        nc.sync.dma_start(out=lvl1p[0:17, m, :, L1W:FTW],
                                          in_=l2[:, :, 8 * m:8 * m + 8])

                    if dbg is not None:
                        nc.sync.dma_start(out=dbg[:, :, :], in_=lvl1)

                    # --- level-1+2 features on packed planes ---
                    ot2 = pool_o2.tile([L1R, 17, FTW], f32, tag="ot2")
                    mueller(pool_t2, FTW,
                            lambda e: lvl1[0:L1R, e],
                            lambda e: lvl1[0:L1R, 16 + e],
                            lambda e: lvl1[0:L1R, 32 + e],
                            lambda k: ot2[0:L1R, k], L1R,
                            xIr=lambda lo, hi: lvl1[0:L1R, lo:hi],
                            oplr=lambda lo, hi: ot2[0:L1R, lo:hi])

                    # unpack lvl2 features -> l2f [17, 17, 32]
                    l2f = pool_o2.tile([L2R, 17, L2W], f32, tag="l2f")
                    ot2p = ot2.rearrange("(r f) k w -> r f k w", f=4)
                    for m in range(4):
                        nc.sync.dma_start(out=l2f[:, :, 8 * m:8 * m + 8],
                                          in_=ot2p[0:17, m, :, L1W:FTW])

                    # --- upsample via PE ---
                    def upsample(nch_base, rloc, wloc, rts, cs, plane_fn):
                        for chn in range(17):
                            plane = plane_fn(chn)
                            pst = pool_ps.tile([128, 128], f32, tag="pst")
                            nc.tensor.transpose(pst[0:wloc, 0:rloc], plane,
                                                ident[0:rloc, 0:rloc])
                            pts = pool_u.tile([L1W, L1R], f32, tag="pts")
                            nc.scalar.copy(pts[0:wloc, 0:rloc],
                                           pst[0:wloc, 0:rloc])
                            psw = pool_ps.tile([L1R, W], f32, tag="psw")
                            nc.tensor.matmul(psw[0:rloc, :], pts[0:wloc, 0:rloc],
                                             cs[0:wloc, :], start=True, stop=True)
                            wres = pool_u.tile([L1R, W], f32, tag="wres")
                            nc.scalar.copy(wres[0:rloc, :], psw[0:rloc, :])
                            for yb in range(2):
                                psf = pool_ps.tile([128, W], f32, tag="psf")
                                nc.tensor.matmul(
                                    psf, rts[0:rloc, yb * 128:(yb + 1) * 128],
                                    wres[0:rloc, :], start=True, stop=True)
                                fin = pool_u.tile([128, W], f32, tag="fin")
                                if chn % 2 == 0:
                                    nc.scalar.copy(fin, psf)
                                else:
                                    nc.vector.tensor_copy(out=fin, in_=psf)
                                nc.sync.dma_start(
                                    out=out[nch_base + chn,
                                            yb * 128:(yb + 1) * 128, :],
                                    in_=fin)

                    upsample(17, L1R, L1W, r1t_s, c1_s,
                             lambda chn: ot2[0:L1R, chn, 0:L1W])
                    upsample(34, L2R, L2W, r2t_s, c2_s,
                             lambda chn: l2f[:, chn, :])

    nc.compile()
    return nc


def kernel(x: np.ndarray) -> np.ndarray:
    from concourse.bass_utils import run_bass_kernel_spmd

    x = np.ascontiguousarray(x, dtype=np.float32)
    B = x.shape[0]
    assert x.shape == (4, CIN, H, W), x.shape

    if "nc" not in _NC_CACHE:
        _NC_CACHE["nc"] = _build_nc()
    nc = _NC_CACHE["nc"]

    consts = [_host_constants(0), _host_constants(1)]
    in_maps = []
    for core in range(N_CORES):
        b, half = core // 2, core % 2
        r1tc, r2tc, c1c, c2c = consts[half]
        if half == 0:
            xm = x[b, :, 0:HALF, :]
            xh = x[b, :, HALF:HALF + HALO, :]
        else:
            xm = x[b, :, HALF:2 * HALF, :]
            xh = x[b, :, HALF - HALO:HALF, :]
        in_maps.append({
            "xmm": np.ascontiguousarray(xm),
            "xhalo": np.ascontiguousarray(xh),
            "r1t": r1tc, "r2t": r2tc, "c1": c1c, "c2": c2c,
        })

    res = run_bass_kernel_spmd(nc, in_maps, core_ids=list(range(N_CORES)))
    outv = np.empty((B, 17 * LEVELS, H, W), np.float32)
    for core in range(N_CORES):
        b, half = core // 2, core % 2
        outv[b, :, half * HALF:(half + 1) * HALF, :] = res.results[core]["out"]
    return outv



# revision 7
# speedup vs baseline: 1.8722x; 1.8722x over previous
"""Mueller-matrix pyramid kernel for Trainium2 (8 NeuronCores).

Sharding: 8 cores = (batch 4) x (H-halves 2). Each core computes the full
51-channel output for its 256-row half at 512 cols.

v2 design (vs the fp32 adjugate baseline):
- fp16 on-chip everywhere: DVE 2-byte tensor_tensor ops run at 2x; the host
  converts inputs to fp16 ([rows, ch, cols] layout so DMA lines are one
  contiguous 48KB run per partition), halving input DMA bytes.
- M = A^-1 I W^-1 via row-normalized Gauss elimination (DVE divide runs at
  the fast rate): ~292 plane-ops/pixel vs ~440 for the adjugate form. The
  right division M W = P is solved as W^T M^T = P^T using transposed plane
  views (free). Batched 4D-AP broadcast products keep instruction counts low.
- Engine split: DVE owns the solves; Pool owns maxpooling + the W-matrix
  internal elimination (independent chain); Act converts fp16->fp32 output
  and drains PSUM; PE does the bilinear upsample in fp16.
- Output written as [rows, 51, 512] fp32 (host transposes back).

SPMD uniformity: 16 halo rows arrive as a separate channels-on-partitions
tensor; per-core R matrices absorb the local->global row permutation.
"""

import numpy as np

H = W = 512
CIN = 48
HALF = 256          # output rows per core
HALO = 16           # extra pooling rows per core
L1R = 68            # local level-1 rows (64 main + 4 halo)
L1W = 128
L2R = 17            # local level-2 rows (16 main + 1 halo)
L2W = 32
PACK2 = 8           # lvl2 packed cols per partition-row (68*8 = 17*32)
FTW = L1W + PACK2   # 136
N_CORES = 8

# ---------------------------------------------------------------------------
# host-side constants
# ---------------------------------------------------------------------------


def _interp_1d(n_out, n_in, lo, hi):
    out = np.zeros((hi - lo, n_in), np.float32)
    scale = (n_in - 1.0) / (n_out - 1.0)
    for i, y in enumerate(range(lo, hi)):
        t = np.float32(y * scale)
        y0 = int(np.floor(t))
        fy = np.float32(t - y0)
        y1 = min(y0 + 1, n_in - 1)
        out[i, y0] += np.float32(1.0) - fy
        out[i, y1] += fy
    return out


def _r_matrix(half, n_in, n_main, off_main, off_halo, n_halo):
    lo, hi = half * HALF, half * HALF + HALF
    full = _interp_1d(H, n_in, lo, hi)
    loc = np.zeros((HALF, n_main + n_halo), np.float32)
    loc[:, :n_main] = full[:, off_main:off_main + n_main]
    loc[:, n_main:] = full[:, off_halo:off_halo + n_halo]
    return loc


def _host_constants(half):
    if half == 0:
        r1 = _r_matrix(0, 128, 64, 0, 64, 4)
        r2 = _r_matrix(0, 32, 16, 0, 16, 1)
    else:
        r1 = _r_matrix(1, 128, 64, 64, 60, 4)
        r2 = _r_matrix(1, 32, 16, 16, 15, 1)
    c1 = _interp_1d(W, L1W, 0, W).T.copy()
    c2 = _interp_1d(W, L2W, 0, W).T.copy()
    return (np.ascontiguousarray(r1.T).astype(np.float16),
            np.ascontiguousarray(r2.T).astype(np.float16),
            np.ascontiguousarray(c1).astype(np.float16),
            np.ascontiguousarray(c2).astype(np.float16))


_NC_CACHE = {}


def _build_nc(repeat=1):
    import concourse.bacc as bacc
    import concourse.mybir as mybir
    from concourse.tile import TileContext
    from concourse.masks import make_identity

    f16 = mybir.dt.float16
    f32 = mybir.dt.float32
    ALU = mybir.AluOpType

    nc = bacc.Bacc("TRN2", target_bir_lowering=False, num_devices=N_CORES)

    xm = nc.dram_tensor("xm", [HALF, CIN, W], f16, kind="ExternalInput")
    xh = nc.dram_tensor("xh", [CIN, HALO, W], f16, kind="ExternalInput")
    r1t = nc.dram_tensor("r1t", [L1R, HALF], f16, kind="ExternalInput")
    r2t = nc.dram_tensor("r2t", [L2R, HALF], f16, kind="ExternalInput")
    c1 = nc.dram_tensor("c1", [L1W, W], f16, kind="ExternalInput")
    c2 = nc.dram_tensor("c2", [L2W, W], f16, kind="ExternalInput")
    out = nc.dram_tensor("out", [HALF, 51, W], f32, kind="ExternalOutput")

    V, G, S = nc.vector, nc.gpsimd, nc.scalar

    def TT(eng, o, a, b, op):
        eng.tensor_tensor(out=o, in0=a, in1=b, op=op)

    def gauss(xB, xBt, xC, sB, sC, sR, rs, FD, eB, eC):
        """Row-normalized Gauss solve of B Y = C, fully in place (Y -> xC).

        xB: [rs,4,4,FD] planes of B; xBt: transposed-index view of the SAME
        planes (xBt[:,v,u] is B[u][v]); xC: [rs,4,4,FD] RHS planes.
        sB/sC: scratch views [rs,3,3,FD] / [rs,3,4,FD]; sR: [rs,4,FD]
        pivot-reciprocal planes (TT divide is not a valid DVE ISA op).
        eB runs the B-internal elimination; eC runs the RHS chain + back-sub.
        """
        for k in range(4):
            piv = xB[:, k:k + 1, k:k + 1, :]
            rk = sR[:, k:k + 1, :].unsqueeze(1)
            V.reciprocal(rk, piv)
            m = 3 - k
            if m:
                rowB = xB[:, k:k + 1, k + 1:, :]
                TT(eB, rowB, rowB, rk.broadcast_to((rs, 1, m, FD)),
                   ALU.mult)
            rowC = xC[:, k:k + 1, :, :]
            TT(eC, rowC, rowC, rk.broadcast_to((rs, 1, 4, FD)), ALU.mult)
            if m:
                colB = xB[:, k + 1:, k:k + 1, :]
                pB = sB[:, 0:m, 0:m, :]
                TT(eB, pB, colB.broadcast_to((rs, m, m, FD)),
                   rowB.broadcast_to((rs, m, m, FD)), ALU.mult)
                TT(eB, xB[:, k + 1:, k + 1:, :], xB[:, k + 1:, k + 1:, :],
                   pB, ALU.subtract)
                pC = sC[:, 0:m, :, :]
                TT(eC, pC, colB.broadcast_to((rs, m, 4, FD)),
                   rowC.broadcast_to((rs, m, 4, FD)), ALU.mult)
                TT(eC, xC[:, k + 1:, :, :], xC[:, k + 1:, :, :], pC,
                   ALU.subtract)
        for i in (2, 1, 0):
            m = 3 - i
            pC = sC[:, 0:m, :, :]
            TT(eC, pC, xBt[:, i + 1:, i:i + 1, :].broadcast_to((rs, m, 4, FD)),
               xC[:, i + 1:, :, :], ALU.mult)
            for j in range(1, m):
                TT(eC, pC[:, 0:1], pC[:, 0:1], pC[:, j:j + 1], ALU.add)
            TT(eC, xC[:, i:i + 1, :, :], xC[:, i:i + 1, :, :], pC[:, 0:1],
               ALU.subtract)

    def solve_views(xt, base):
        nat = xt[:, base:base + 16].rearrange("p (i j) w -> p i j w", j=4)
        tr = xt[:, base:base + 16].rearrange("p (i j) w -> p j i w", j=4)
        return nat, tr

    with nc.allow_low_precision(reason="fp16 pipeline; tol 2e-2"), \
            TileContext(nc) as tc:
        with (
            tc.tile_pool(name="cst", bufs=1) as pool_c,
            tc.tile_pool(name="lvl", bufs=1) as pool_l,
        ):
            ident = pool_c.tile([128, 128], f16)
            make_identity(nc, ident)
            r1t_s = pool_c.tile([L1R, HALF], f16)
            nc.sync.dma_start(out=r1t_s, in_=r1t[:, :])
            r2t_s = pool_c.tile([L2R, HALF], f16)
            nc.sync.dma_start(out=r2t_s, in_=r2t[:, :])
            c1_s = pool_c.tile([L1W, W], f16)
            nc.sync.dma_start(out=c1_s, in_=c1[:, :])
            c2_s = pool_c.tile([L2W, W], f16)
            nc.sync.dma_start(out=c2_s, in_=c2[:, :])

            for _rep in range(repeat):
                lvl1 = pool_l.tile([L1R, CIN, FTW], f16)

                # ============ phase 0: level-0 tiles (rows on partitions) ====
                with (
                    tc.tile_pool(name="px", bufs=2) as pool_x,
                    tc.tile_pool(name="pw", bufs=1) as pool_w,
                    tc.tile_pool(name="ps", bufs=1) as pool_s,
                    tc.tile_pool(name="po", bufs=1) as pool_o,
                ):
                    xts = []
                    for rt in range(2):
                        xt = pool_x.tile([128, CIN, W], f16, tag="xt")
                        nc.sync.dma_start(
                            out=xt, in_=xm[rt * 128:(rt + 1) * 128, :, :])
                        xts.append(xt)

                    wp = pool_w.tile([128, 2, CIN, L1W], f16, tag="wp")
                    sBt = pool_s.tile([128, 9, W], f16, tag="sB")
                    sCt = pool_s.tile([128, 12, W], f16, tag="sC")
                    sSum = pool_s.tile([128, 1, W], f16, tag="sSum")
                    sR = pool_s.tile([128, 4, W], f16, tag="sR")
                    otf = pool_o.tile([128, 17, W // 2], f32, tag="otf")
                    sB = sBt.rearrange("p (a b) w -> p a b w", b=3)
                    sC = sCt.rearrange("p (a b) w -> p a b w", b=4)

                    for rt in range(2):
                        xt = xts[rt]
                        # DVE: W-pool; cols arrive phase-deinterleaved
                        # ((f,q) layout) so all operands are packed -> fast
                        xv = xt.rearrange("p c (f q) -> p c f q", q=128)
                        wps = wp[:, rt]
                        TT(V, wps, xv[:, :, 0], xv[:, :, 1], ALU.max)
                        TT(V, wps, wps, xv[:, :, 2], ALU.max)
                        TT(V, wps, wps, xv[:, :, 3], ALU.max)

                        # Pool: intensity tree (sum -> sSum; scaled later)
                        TT(G, sCt[:, 0:8], xt[:, 0:8], xt[:, 8:16], ALU.add)
                        TT(G, sCt[:, 0:4], sCt[:, 0:4], sCt[:, 4:8], ALU.add)
                        TT(G, sCt[:, 0:2], sCt[:, 0:2], sCt[:, 2:4], ALU.add)
                        TT(G, sSum[:, 0], sCt[:, 0], sCt[:, 1], ALU.add)

                        xi4, xi4t = solve_views(xt, 0)
                        xa4, xa4t = solve_views(xt, 16)
                        xw4, xw4t = solve_views(xt, 32)
                        # X = A^-1 I  (all DVE)
                        gauss(xa4, xa4t, xi4, sB, sC, sR, 128, W, G, V)
                        # M^T = (W^T)^-1 X^T  (B-elim on Pool, RHS on DVE)
                        gauss(xw4t, xw4, xi4t, sB, sC, sR, 128, W, G, V)
                        # normalize M by M00 into the dead A planes (fp16)
                        V.reciprocal(sR[:, 0:1, :], xt[:, 0:1])
                        TT(V, xt[:, 16:32], xt[:, 0:16],
                           sR[:, 0:1, :].broadcast_to((128, 16, W)),
                           ALU.mult)
                        # fp32 conversion + output staging (col halves
                        # to halve the f32 staging tile)
                        sSv = sSum[:, 0].rearrange("p (f q) -> p f q",
                                                          q=128)
                        mmv = xt[:, 16:32].rearrange("p c (f q) -> p c f q",
                                                     q=128)
                        for cw in range(2):
                            qs = slice(cw * 64, cw * 64 + 64)
                            V.tensor_scalar(
                                out=otf[:, 0].rearrange("p (q f) -> p f q",
                                                        f=4),
                                in0=sSv[:, :, qs],
                                scalar1=1.0 / 16.0, scalar2=None,
                                op0=ALU.mult)
                            S.copy(otf[:, 1:17].rearrange(
                                "p c (q f) -> p c f q", f=4),
                                mmv[:, :, :, qs])
                            nc.sync.dma_start(
                                out=out[rt * 128:(rt + 1) * 128, 0:17,
                                        cw * 256:cw * 256 + 256],
                                in_=otf)

                    # H-pool: gather partition-strided rows, max into lvl1
                    wpr = wp.rearrange("(r f) t c w -> r f t c w", f=4)
                    for t in range(2):
                        nc.sync.dma_start(out=lvl1[32 * t:32 * t + 32, :,
                                                   0:L1W],
                                          in_=wpr[:, 0, t])
                    gt = pool_w.tile([64, CIN, L1W], f16, tag="gt")
                    for dy in (1, 2, 3):
                        for t in range(2):
                            nc.sync.dma_start(out=gt[32 * t:32 * t + 32],
                                              in_=wpr[:, dy, t])
                        TT(V, lvl1[0:64, :, 0:L1W], lvl1[0:64, :, 0:L1W], gt,
                           ALU.max)

                # ============ phase 1: halo, levels 1+2, upsample ============
                with (
                    tc.tile_pool(name="ph", bufs=1) as pool_h,
                    tc.tile_pool(name="p2", bufs=1) as pool_2,
                    tc.tile_pool(name="pu", bufs=1) as pool_u,
                    tc.tile_pool(name="pup", bufs=2) as pool_up,
                    tc.tile_pool(name="psw", bufs=2, space="PSUM") as pool_pw,
                    tc.tile_pool(name="psf", bufs=1, space="PSUM") as pool_pf,
                ):
                    # halo pooling, channels on partitions
                    xht = pool_h.tile([CIN, HALO, W], f16, tag="xht")
                    nc.sync.dma_start(out=xht, in_=xh[:, :, :])
                    hw_ = pool_h.tile([CIN, HALO, L1W], f16, tag="hw")
                    xh4 = xht.rearrange("c r (f q) -> c r f q", q=128)
                    TT(V, hw_, xh4[:, :, 0], xh4[:, :, 1], ALU.max)
                    TT(V, hw_, hw_, xh4[:, :, 2], ALU.max)
                    TT(V, hw_, hw_, xh4[:, :, 3], ALU.max)
                    hp = pool_h.tile([CIN, 4, L1W], f16, tag="hp")
                    hv = hw_.rearrange("c (g r) w -> c g r w", r=4)
                    TT(V, hp, hv[:, :, 0], hv[:, :, 1], ALU.max)
                    TT(V, hp, hp, hv[:, :, 2], ALU.max)
                    TT(V, hp, hp, hv[:, :, 3], ALU.max)
                    for g in range(4):
                        nc.sync.dma_start(out=lvl1[64 + g:65 + g, :, 0:L1W],
                                          in_=hp[:, g:g + 1, :])

                    # level-2 pooling from lvl1 cols 0:128
                    wp2 = pool_2.tile([L1R, CIN, L2W], f16, tag="wp2")
                    l14 = lvl1[:, :, 0:L1W].rearrange("p c (w f) -> p c w f",
                                                      f=4)
                    TT(V, wp2, l14[:, :, :, 0], l14[:, :, :, 1], ALU.max)
                    TT(V, wp2, wp2, l14[:, :, :, 2], ALU.max)
                    TT(V, wp2, wp2, l14[:, :, :, 3], ALU.max)
                    wp2r = wp2.rearrange("(r f) c w -> r f c w", f=4)
                    l2 = pool_2.tile([L2R, CIN, L2W], f16, tag="l2")
                    nc.sync.dma_start(out=l2, in_=wp2r[:, 0])
                    g2 = pool_2.tile([L2R, CIN, L2W], f16, tag="g2")
                    for dy in (1, 2, 3):
                        nc.sync.dma_start(out=g2, in_=wp2r[:, dy])
                        TT(V, l2, l2, g2, ALU.max)
                    # pack lvl2 into lvl1 cols 128:136
                    lvl1p = lvl1.rearrange("(r f) c w -> r f c w", f=4)
                    for m in range(4):
                        nc.sync.dma_start(out=lvl1p[0:17, m, :, L1W:FTW],
                                          in_=l2[:, :, 8 * m:8 * m + 8])

                    # level-1+2 features on packed [68, 48, 136] planes
                    sBt2 = pool_2.tile([L1R, 9, FTW], f16, tag="sB2")
                    sCt2 = pool_2.tile([L1R, 12, FTW], f16, tag="sC2")
                    sSum2 = pool_2.tile([L1R, 1, FTW], f16, tag="sSum2")
                    sR2 = pool_2.tile([L1R, 4, FTW], f16, tag="sR2")
                    ot2 = pool_2.tile([L1R, 17, FTW], f16, tag="ot2")
                    sB2 = sBt2.rearrange("p (a b) w -> p a b w", b=3)
                    sC2 = sCt2.rearrange("p (a b) w -> p a b w", b=4)

                    TT(G, sCt2[:, 0:8], lvl1[:, 0:8], lvl1[:, 8:16],
                       ALU.add)
                    TT(G, sCt2[:, 0:4], sCt2[:, 0:4], sCt2[:, 4:8], ALU.add)
                    TT(G, sCt2[:, 0:2], sCt2[:, 0:2], sCt2[:, 2:4], ALU.add)
                    TT(G, sSum2[:, 0], sCt2[:, 0], sCt2[:, 1], ALU.add)
                    V.tensor_scalar(out=ot2[:, 0], in0=sSum2[:, 0],
                                    scalar1=1.0 / 16.0, scalar2=None,
                                    op0=ALU.mult)

                    li4, li4t = solve_views(lvl1, 0)
                    la4, la4t = solve_views(lvl1, 16)
                    lw4, lw4t = solve_views(lvl1, 32)
                    gauss(la4, la4t, li4, sB2, sC2, sR2, L1R, FTW, G, V)
                    gauss(lw4t, lw4, li4t, sB2, sC2, sR2, L1R, FTW, G, V)
                    V.reciprocal(sR2[:, 0:1, :], lvl1[:, 0:1])
                    TT(V, ot2[:, 1:17], lvl1[:, 0:16],
                       sR2[:, 0:1, :].broadcast_to((L1R, 16, FTW)),
                       ALU.mult)

                    # unpack lvl2 features -> l2f [17, 17, 32]
                    l2f = pool_2.tile([L2R, 17, L2W], f16, tag="l2f")
                    ot2p = ot2.rearrange("(r f) k w -> r f k w", f=4)
                    for m in range(4):
                        nc.sync.dma_start(out=l2f[:, :, 8 * m:8 * m + 8],
                                          in_=ot2p[0:17, m, :, L1W:FTW])

                    # --- bilinear upsample via PE (fp16 matmuls) ---
                    def upsample(nch_base, rloc, wloc, rts, cs, plane_fn):
                        groups = [(0, 4), (4, 4), (8, 4), (12, 4), (16, 1)]
                        for g0, gn in groups:
                            wrs = []
                            for gi in range(gn):
                                chn = g0 + gi
                                pst = pool_pw.tile([128, 128], f16,
                                                   tag="pst")
                                nc.tensor.transpose(pst[0:wloc, 0:rloc],
                                                    plane_fn(chn),
                                                    ident[0:rloc, 0:rloc])
                                pts = pool_up.tile([L1W, L1R], f16, tag="pts")
                                S.copy(pts[0:wloc, 0:rloc],
                                       pst[0:wloc, 0:rloc])
                                psw = pool_pw.tile([L1R, W], f32, tag="psw")
                                nc.tensor.matmul(psw[0:rloc, :],
                                                 pts[0:wloc, 0:rloc],
                                                 cs[0:wloc, :],
                                                 start=True, stop=True)
                                wres = pool_up.tile([L1R, W], f16,
                                                    tag=f"wres{gi}")
                                S.copy(wres[0:rloc, :], psw[0:rloc, :])
                                wrs.append(wres)
                            psf = pool_pf.tile([128, 4, W], f32, tag="psf")
                            for yb in range(2):
                                for gi in range(gn):
                                    nc.tensor.matmul(
                                        psf[:, gi, :],
                                        rts[0:rloc,
                                            yb * 128:(yb + 1) * 128],
                                        wrs[gi][0:rloc, :],
                                        start=True, stop=True)
                                fin = pool_up.tile([128, 4, W], f32,
                                                   tag="fin")
                                S.copy(fin[:, 0:gn], psf[:, 0:gn])
                                nc.sync.dma_start(
                                    out=out[yb * 128:(yb + 1) * 128,
                                            nch_base + g0:nch_base + g0 + gn,
                                            :],
                                    in_=fin[:, 0:gn])

                    upsample(17, L1R, L1W, r1t_s, c1_s,
                             lambda chn: ot2[0:L1R, chn, 0:L1W])
                    upsample(34, L2R, L2W, r2t_s, c2_s,
                             lambda chn: l2f[0:L2R, chn, 0:L2W])

    nc.compile()
    return nc


def kernel(x: np.ndarray) -> np.ndarray:
    from concourse.bass_utils import run_bass_kernel_spmd

    assert x.shape == (4, CIN, H, W), x.shape
    x16 = np.ascontiguousarray(x, dtype=np.float32).astype(np.float16)
    B = x.shape[0]

    if "nc" not in _NC_CACHE:
        _NC_CACHE["nc"] = _build_nc()
    nc = _NC_CACHE["nc"]

    consts = [_host_constants(0), _host_constants(1)]
    in_maps = []
    for core in range(N_CORES):
        b, half = core // 2, core % 2
        r1tc, r2tc, c1c, c2c = consts[half]
        r0 = half * HALF
        xmv = x16[b, :, r0:r0 + HALF, :].transpose(1, 0, 2)
        xmv = np.ascontiguousarray(
            xmv.reshape(HALF, CIN, L1W, 4).transpose(0, 1, 3, 2)
            .reshape(HALF, CIN, W))
        if half == 0:
            xhv = x16[b, :, HALF:HALF + HALO, :]
        else:
            xhv = x16[b, :, HALF - HALO:HALF, :]
        xhv = np.ascontiguousarray(
            xhv.reshape(CIN, HALO, L1W, 4).transpose(0, 1, 3, 2)
            .reshape(CIN, HALO, W))
        in_maps.append({
            "xm": xmv, "xh": xhv,
            "r1t": r1tc, "r2t": r2tc, "c1": c1c, "c2": c2c,
        })

    res = run_bass_kernel_spmd(nc, in_maps, core_ids=list(range(N_CORES)))
    outv = np.empty((B, 17 * 3, H, W), np.float32)
    for core in range(N_CORES):
        b, half = core // 2, core % 2
        outv[b, :, half * HALF:(half + 1) * HALF, :] = \
            res.results[core]["out"].transpose(1, 0, 2)
    return outv


# revision 8
# speedup vs baseline: 2.2239x; 1.1878x over previous
"""Mueller-matrix pyramid kernel for Trainium2 (8 NeuronCores).

Sharding: 8 cores = (batch 4) x (H-halves 2). Each core computes the full
51-channel output for its 256-row half at 512 cols.

v2 design (vs the fp32 adjugate baseline):
- fp16 on-chip everywhere: DVE 2-byte tensor_tensor ops run at 2x; the host
  converts inputs to fp16 ([rows, ch, cols] layout so DMA lines are one
  contiguous 48KB run per partition), halving input DMA bytes.
- M = A^-1 I W^-1 via row-normalized Gauss elimination (DVE divide runs at
  the fast rate): ~292 plane-ops/pixel vs ~440 for the adjugate form. The
  right division M W = P is solved as W^T M^T = P^T using transposed plane
  views (free). Batched 4D-AP broadcast products keep instruction counts low.
- Engine split: DVE owns the solves; Pool owns maxpooling + the W-matrix
  internal elimination (independent chain); Act converts fp16->fp32 output
  and drains PSUM; PE does the bilinear upsample in fp16.
- Output written as [rows, 51, 512] fp32 (host transposes back).

SPMD uniformity: 16 halo rows arrive as a separate channels-on-partitions
tensor; per-core R matrices absorb the local->global row permutation.
"""

import numpy as np

H = W = 512
CIN = 48
HALF = 256          # output rows per core
HALO = 16           # extra pooling rows per core
L1R = 68            # local level-1 rows (64 main + 4 halo)
L1W = 128
L2R = 17            # local level-2 rows (16 main + 1 halo)
L2W = 32
PACK2 = 8           # lvl2 packed cols per partition-row (68*8 = 17*32)
FTW = L1W + PACK2   # 136
N_CORES = 8

# ---------------------------------------------------------------------------
# host-side constants
# ---------------------------------------------------------------------------


def _interp_1d(n_out, n_in, lo, hi):
    out = np.zeros((hi - lo, n_in), np.float32)
    scale = (n_in - 1.0) / (n_out - 1.0)
    for i, y in enumerate(range(lo, hi)):
        t = np.float32(y * scale)
        y0 = int(np.floor(t))
        fy = np.float32(t - y0)
        y1 = min(y0 + 1, n_in - 1)
        out[i, y0] += np.float32(1.0) - fy
        out[i, y1] += fy
    return out


def _r_matrix(half, n_in, n_main, off_main, off_halo, n_halo):
    lo, hi = half * HALF, half * HALF + HALF
    full = _interp_1d(H, n_in, lo, hi)
    loc = np.zeros((HALF, n_main + n_halo), np.float32)
    loc[:, :n_main] = full[:, off_main:off_main + n_main]
    loc[:, n_main:] = full[:, off_halo:off_halo + n_halo]
    return loc


def _host_constants(half):
    if half == 0:
        r1 = _r_matrix(0, 128, 64, 0, 64, 4)
        r2 = _r_matrix(0, 32, 16, 0, 16, 1)
    else:
        r1 = _r_matrix(1, 128, 64, 64, 60, 4)
        r2 = _r_matrix(1, 32, 16, 16, 15, 1)
    c1 = _interp_1d(W, L1W, 0, W).T.copy()
    c2 = _interp_1d(W, L2W, 0, W).T.copy()
    return (np.ascontiguousarray(r1.T).astype(np.float16),
            np.ascontiguousarray(r2.T).astype(np.float16),
            np.ascontiguousarray(c1).astype(np.float16),
            np.ascontiguousarray(c2).astype(np.float16))


_NC_CACHE = {}


def _build_nc(repeat=1):
    import concourse.bacc as bacc
    import concourse.mybir as mybir
    from concourse.tile import TileContext
    from concourse.masks import make_identity

    f16 = mybir.dt.float16
    f32 = mybir.dt.float32
    ALU = mybir.AluOpType

    nc = bacc.Bacc("TRN2", target_bir_lowering=False, num_devices=N_CORES)

    xm = nc.dram_tensor("xm", [HALF, CIN, W], f16, kind="ExternalInput")
    xh = nc.dram_tensor("xh", [CIN, HALO, W], f16, kind="ExternalInput")
    r1t = nc.dram_tensor("r1t", [L1R, HALF], f16, kind="ExternalInput")
    r2t = nc.dram_tensor("r2t", [L2R, HALF], f16, kind="ExternalInput")
    c1 = nc.dram_tensor("c1", [L1W, W], f16, kind="ExternalInput")
    c2 = nc.dram_tensor("c2", [L2W, W], f16, kind="ExternalInput")
    out = nc.dram_tensor("out", [HALF, 51, W], f32, kind="ExternalOutput")

    V, G, S = nc.vector, nc.gpsimd, nc.scalar

    def TT(eng, o, a, b, op):
        eng.tensor_tensor(out=o, in0=a, in1=b, op=op)

    def gauss(xB, xBt, xC, sB, sC, sP, sR, rs, FD):
        """Row-normalized Gauss solve of B Y = C, fully in place (Y -> xC).

        xB: [rs,4,4,FD] planes of B; xBt: transposed-index view of the SAME
        planes (xBt[:,v,u] is B[u][v]); xC: [rs,4,4,FD] RHS planes.
        Engine split with ONE-WAY cross-engine flow: DVE runs the B
        elimination, the reciprocals (sR) and RHS columns 0:3; Pool runs RHS
        column 3 end-to-end (it only consumes DVE outputs, never gates it).
        sB/sC: DVE scratch [rs,3,3,FD]; sP: Pool scratch [rs,3,1,FD].
        """
        for k in range(4):
            piv = xB[:, k:k + 1, k:k + 1, :]
            rk = sR[:, k:k + 1, :].unsqueeze(1)
            V.reciprocal(rk, piv)
            m = 3 - k
            if m:
                rowB = xB[:, k:k + 1, k + 1:, :]
                TT(V, rowB, rowB, rk.broadcast_to((rs, 1, m, FD)), ALU.mult)
            rowCv = xC[:, k:k + 1, 0:3, :]
            TT(V, rowCv, rowCv, rk.broadcast_to((rs, 1, 3, FD)), ALU.mult)
            rowCp = xC[:, k:k + 1, 3:4, :]
            TT(G, rowCp, rowCp, rk.broadcast_to((rs, 1, 1, FD)), ALU.mult)
            if m:
                colB = xB[:, k + 1:, k:k + 1, :]
                pB = sB[:, 0:m, 0:m, :]
                TT(V, pB, colB.broadcast_to((rs, m, m, FD)),
                   rowB.broadcast_to((rs, m, m, FD)), ALU.mult)
                TT(V, xB[:, k + 1:, k + 1:, :], xB[:, k + 1:, k + 1:, :],
                   pB, ALU.subtract)
                pC = sC[:, 0:m, :, :]
                TT(V, pC, colB.broadcast_to((rs, m, 3, FD)),
                   rowCv.broadcast_to((rs, m, 3, FD)), ALU.mult)
                TT(V, xC[:, k + 1:, 0:3, :], xC[:, k + 1:, 0:3, :], pC,
                   ALU.subtract)
                pP = sP[:, 0:m, :, :]
                TT(G, pP, colB.broadcast_to((rs, m, 1, FD)),
                   rowCp.broadcast_to((rs, m, 1, FD)), ALU.mult)
                TT(G, xC[:, k + 1:, 3:4, :], xC[:, k + 1:, 3:4, :], pP,
                   ALU.subtract)
        for i in (2, 1, 0):
            m = 3 - i
            urow = xBt[:, i + 1:, i:i + 1, :]
            pC = sC[:, 0:m, :, :]
            TT(V, pC, urow.broadcast_to((rs, m, 3, FD)),
               xC[:, i + 1:, 0:3, :], ALU.mult)
            for j in range(1, m):
                TT(V, pC[:, 0:1], pC[:, 0:1], pC[:, j:j + 1], ALU.add)
            TT(V, xC[:, i:i + 1, 0:3, :], xC[:, i:i + 1, 0:3, :], pC[:, 0:1],
               ALU.subtract)
            pP = sP[:, 0:m, :, :]
            TT(G, pP, urow.broadcast_to((rs, m, 1, FD)),
               xC[:, i + 1:, 3:4, :], ALU.mult)
            for j in range(1, m):
                TT(G, pP[:, 0:1], pP[:, 0:1], pP[:, j:j + 1], ALU.add)
            TT(G, xC[:, i:i + 1, 3:4, :], xC[:, i:i + 1, 3:4, :], pP[:, 0:1],
               ALU.subtract)

    def solve_views(xt, base):
        nat = xt[:, base:base + 16].rearrange("p (i j) w -> p i j w", j=4)
        tr = xt[:, base:base + 16].rearrange("p (i j) w -> p j i w", j=4)
        return nat, tr

    with nc.allow_low_precision(reason="fp16 pipeline; tol 2e-2"), \
            TileContext(nc) as tc:
        with (
            tc.tile_pool(name="cst", bufs=1) as pool_c,
            tc.tile_pool(name="lvl", bufs=1) as pool_l,
        ):
            ident = pool_c.tile([128, 128], f16)
            make_identity(nc, ident)
            r1t_s = pool_c.tile([L1R, HALF], f16)
            nc.sync.dma_start(out=r1t_s, in_=r1t[:, :])
            r2t_s = pool_c.tile([L2R, HALF], f16)
            nc.sync.dma_start(out=r2t_s, in_=r2t[:, :])
            c1_s = pool_c.tile([L1W, W], f16)
            nc.sync.dma_start(out=c1_s, in_=c1[:, :])
            c2_s = pool_c.tile([L2W, W], f16)
            nc.sync.dma_start(out=c2_s, in_=c2[:, :])

            for _rep in range(repeat):
                lvl1 = pool_l.tile([L1R, CIN, FTW], f16)

                # ============ phase 0: level-0 tiles (rows on partitions) ====
                with (
                    tc.tile_pool(name="px", bufs=2) as pool_x,
                    tc.tile_pool(name="pw", bufs=1) as pool_w,
                    tc.tile_pool(name="ps", bufs=1) as pool_s,
                    tc.tile_pool(name="po", bufs=1) as pool_o,
                ):
                    xts = []
                    for rt in range(2):
                        xt = pool_x.tile([128, CIN, W], f16, tag="xt")
                        nc.sync.dma_start(
                            out=xt, in_=xm[rt * 128:(rt + 1) * 128, :, :])
                        xts.append(xt)

                    wp = pool_w.tile([128, 2, CIN, L1W], f16, tag="wp")
                    sBt = pool_s.tile([128, 9, W], f16, tag="sB")
                    sCt = pool_s.tile([128, 9, W], f16, tag="sC")
                    sPt = pool_s.tile([128, 3, W], f16, tag="sP")
                    sIt = pool_s.tile([128, 8, W], f16, tag="sI")
                    sSum = pool_s.tile([128, 2, W], f16, tag="sSum")
                    sRa = pool_s.tile([128, 4, W], f16, tag="sRa")
                    sRw = pool_s.tile([128, 4, W], f16, tag="sRw")
                    otf = pool_o.tile([128, 17, W // 2], f32, tag="otf")
                    sB = sBt.rearrange("p (a b) w -> p a b w", b=3)
                    sC = sCt.rearrange("p (a b) w -> p a b w", b=3)
                    sP = sPt.rearrange("p (a b) w -> p a b w", b=1)

                    for rt in range(2):
                        xt = xts[rt]
                        # DVE: W-pool; cols arrive phase-deinterleaved
                        # ((f,q) layout) so all operands are packed -> fast
                        xv = xt.rearrange("p c (f q) -> p c f q", q=128)
                        wps = wp[:, rt]
                        TT(V, wps, xv[:, :, 0], xv[:, :, 1], ALU.max)
                        TT(V, wps, wps, xv[:, :, 2], ALU.max)
                        TT(V, wps, wps, xv[:, :, 3], ALU.max)

                        # Pool: intensity tree (sum -> sSum; scaled later)
                        TT(G, sIt[:, 0:8], xt[:, 0:8], xt[:, 8:16], ALU.add)
                        TT(G, sIt[:, 0:4], sIt[:, 0:4], sIt[:, 4:8], ALU.add)
                        TT(G, sIt[:, 0:2], sIt[:, 0:2], sIt[:, 2:4], ALU.add)
                        TT(G, sSum[:, rt], sIt[:, 0], sIt[:, 1], ALU.add)

                        xi4, xi4t = solve_views(xt, 0)
                        xa4, xa4t = solve_views(xt, 16)
                        xw4, xw4t = solve_views(xt, 32)
                        # X = A^-1 I  (all DVE)
                        gauss(xa4, xa4t, xi4, sB, sC, sP, sRa, 128, W)
                        # M^T = (W^T)^-1 X^T
                        gauss(xw4t, xw4, xi4t, sB, sC, sP, sRw, 128, W)
                        # normalize M by M00 into the dead A planes (fp16);
                        # Pool runs the big multiply (one-way: V recip -> G)
                        V.reciprocal(sRa[:, 0:1, :], xt[:, 0:1])
                        TT(G, xt[:, 16:32], xt[:, 0:16],
                           sRa[:, 0:1, :].broadcast_to((128, 16, W)),
                           ALU.mult)
                        # fp32 conversion + output staging (col halves
                        # to halve the f32 staging tile)
                        sSv = sSum[:, rt].rearrange("p (f q) -> p f q",
                                                           q=128)
                        mmv = xt[:, 16:32].rearrange("p c (f q) -> p c f q",
                                                     q=128)
                        for cw in range(2):
                            qs = slice(cw * 64, cw * 64 + 64)
                            V.tensor_scalar(
                                out=otf[:, 0].rearrange("p (q f) -> p f q",
                                                        f=4),
                                in0=sSv[:, :, qs],
                                scalar1=1.0 / 16.0, scalar2=None,
                                op0=ALU.mult)
                            S.copy(otf[:, 1:17].rearrange(
                                "p c (q f) -> p c f q", f=4),
                                mmv[:, :, :, qs])
                            nc.sync.dma_start(
                                out=out[rt * 128:(rt + 1) * 128, 0:17,
                                        cw * 256:cw * 256 + 256],
                                in_=otf)

                    # H-pool: gather partition-strided rows, max into lvl1
                    wpr = wp.rearrange("(r f) t c w -> r f t c w", f=4)
                    for t in range(2):
                        nc.sync.dma_start(out=lvl1[32 * t:32 * t + 32, :,
                                                   0:L1W],
                                          in_=wpr[:, 0, t])
                    gt = pool_w.tile([64, CIN, L1W], f16, tag="gt")
                    for dy in (1, 2, 3):
                        for t in range(2):
                            nc.sync.dma_start(out=gt[32 * t:32 * t + 32],
                                              in_=wpr[:, dy, t])
                        TT(V, lvl1[0:64, :, 0:L1W], lvl1[0:64, :, 0:L1W], gt,
                           ALU.max)

                # ============ phase 1: halo, levels 1+2, upsample ============
                with (
                    tc.tile_pool(name="ph", bufs=1) as pool_h,
                    tc.tile_pool(name="p2", bufs=1) as pool_2,
                    tc.tile_pool(name="pu", bufs=1) as pool_u,
                    tc.tile_pool(name="pup", bufs=2) as pool_up,
                    tc.tile_pool(name="psw", bufs=2, space="PSUM") as pool_pw,
                    tc.tile_pool(name="psf", bufs=1, space="PSUM") as pool_pf,
                ):
                    # halo pooling, channels on partitions
                    xht = pool_h.tile([CIN, HALO, W], f16, tag="xht")
                    nc.sync.dma_start(out=xht, in_=xh[:, :, :])
                    hw_ = pool_h.tile([CIN, HALO, L1W], f16, tag="hw")
                    xh4 = xht.rearrange("c r (f q) -> c r f q", q=128)
                    TT(V, hw_, xh4[:, :, 0], xh4[:, :, 1], ALU.max)
                    TT(V, hw_, hw_, xh4[:, :, 2], ALU.max)
                    TT(V, hw_, hw_, xh4[:, :, 3], ALU.max)
                    hp = pool_h.tile([CIN, 4, L1W], f16, tag="hp")
                    hv = hw_.rearrange("c (g r) w -> c g r w", r=4)
                    TT(V, hp, hv[:, :, 0], hv[:, :, 1], ALU.max)
                    TT(V, hp, hp, hv[:, :, 2], ALU.max)
                    TT(V, hp, hp, hv[:, :, 3], ALU.max)
                    for g in range(4):
                        nc.sync.dma_start(out=lvl1[64 + g:65 + g, :, 0:L1W],
                                          in_=hp[:, g:g + 1, :])

                    # level-2 pooling from lvl1 cols 0:128
                    wp2 = pool_2.tile([L1R, CIN, L2W], f16, tag="wp2")
                    l14 = lvl1[:, :, 0:L1W].rearrange("p c (w f) -> p c w f",
                                                      f=4)
                    TT(V, wp2, l14[:, :, :, 0], l14[:, :, :, 1], ALU.max)
                    TT(V, wp2, wp2, l14[:, :, :, 2], ALU.max)
                    TT(V, wp2, wp2, l14[:, :, :, 3], ALU.max)
                    wp2r = wp2.rearrange("(r f) c w -> r f c w", f=4)
                    l2 = pool_2.tile([L2R, CIN, L2W], f16, tag="l2")
                    nc.sync.dma_start(out=l2, in_=wp2r[:, 0])
                    g2 = pool_2.tile([L2R, CIN, L2W], f16, tag="g2")
                    for dy in (1, 2, 3):
                        nc.sync.dma_start(out=g2, in_=wp2r[:, dy])
                        TT(V, l2, l2, g2, ALU.max)
                    # pack lvl2 into lvl1 cols 128:136
                    lvl1p = lvl1.rearrange("(r f) c w -> r f c w", f=4)
                    for m in range(4):
                        nc.sync.dma_start(out=lvl1p[0:17, m, :, L1W:FTW],
                                          in_=l2[:, :, 8 * m:8 * m + 8])

                    # level-1+2 features on packed [68, 48, 136] planes
                    sBt2 = pool_2.tile([L1R, 9, FTW], f16, tag="sB2")
                    sCt2 = pool_2.tile([L1R, 9, FTW], f16, tag="sC2")
                    sPt2 = pool_2.tile([L1R, 3, FTW], f16, tag="sP2")
                    sIt2 = pool_2.tile([L1R, 8, FTW], f16, tag="sI2")
                    sSum2 = pool_2.tile([L1R, 1, FTW], f16, tag="sSum2")
                    sRa2 = pool_2.tile([L1R, 4, FTW], f16, tag="sRa2")
                    sRw2 = pool_2.tile([L1R, 4, FTW], f16, tag="sRw2")
                    ot2 = pool_2.tile([L1R, 17, FTW], f16, tag="ot2")
                    sB2 = sBt2.rearrange("p (a b) w -> p a b w", b=3)
                    sC2 = sCt2.rearrange("p (a b) w -> p a b w", b=3)
                    sP2 = sPt2.rearrange("p (a b) w -> p a b w", b=1)

                    TT(G, sIt2[:, 0:8], lvl1[:, 0:8], lvl1[:, 8:16],
                       ALU.add)
                    TT(G, sIt2[:, 0:4], sIt2[:, 0:4], sIt2[:, 4:8], ALU.add)
                    TT(G, sIt2[:, 0:2], sIt2[:, 0:2], sIt2[:, 2:4], ALU.add)
                    TT(G, sSum2[:, 0], sIt2[:, 0], sIt2[:, 1], ALU.add)
                    V.tensor_scalar(out=ot2[:, 0], in0=sSum2[:, 0],
                                    scalar1=1.0 / 16.0, scalar2=None,
                                    op0=ALU.mult)

                    li4, li4t = solve_views(lvl1, 0)
                    la4, la4t = solve_views(lvl1, 16)
                    lw4, lw4t = solve_views(lvl1, 32)
                    gauss(la4, la4t, li4, sB2, sC2, sP2, sRa2, L1R, FTW)
                    gauss(lw4t, lw4, li4t, sB2, sC2, sP2, sRw2, L1R, FTW)
                    V.reciprocal(sRa2[:, 0:1, :], lvl1[:, 0:1])
                    TT(G, ot2[:, 1:17], lvl1[:, 0:16],
                       sRa2[:, 0:1, :].broadcast_to((L1R, 16, FTW)),
                       ALU.mult)

                    # unpack lvl2 features -> l2f [17, 17, 32]
                    l2f = pool_2.tile([L2R, 17, L2W], f16, tag="l2f")
                    ot2p = ot2.rearrange("(r f) k w -> r f k w", f=4)
                    for m in range(4):
                        nc.sync.dma_start(out=l2f[:, :, 8 * m:8 * m + 8],
                                          in_=ot2p[0:17, m, :, L1W:FTW])

                    # --- bilinear upsample via PE (fp16 matmuls) ---
                    def upsample(nch_base, rloc, wloc, rts, cs, plane_fn):
                        groups = [(0, 4), (4, 4), (8, 4), (12, 4), (16, 1)]
                        for g0, gn in groups:
                            wrs = []
                            for gi in range(gn):
                                chn = g0 + gi
                                pst = pool_pw.tile([128, 128], f16,
                                                   tag="pst")
                                nc.tensor.transpose(pst[0:wloc, 0:rloc],
                                                    plane_fn(chn),
                                                    ident[0:rloc, 0:rloc])
                                pts = pool_up.tile([L1W, L1R], f16, tag="pts")
                                S.copy(pts[0:wloc, 0:rloc],
                                       pst[0:wloc, 0:rloc])
                                psw = pool_pw.tile([L1R, W], f32, tag="psw")
                                nc.tensor.matmul(psw[0:rloc, :],
                                                 pts[0:wloc, 0:rloc],
                                                 cs[0:wloc, :],
                                                 start=True, stop=True)
                                wres = pool_up.tile([L1R, W], f16,
                                                    tag=f"wres{gi}")
                                S.copy(wres[0:rloc, :], psw[0:rloc, :])
                                wrs.append(wres)
                            psf = pool_pf.tile([128, 4, W], f32, tag="psf")
                            for yb in range(2):
                                for gi in range(gn):
                                    nc.tensor.matmul(
                                        psf[:, gi, :],
                                        rts[0:rloc,
                                            yb * 128:(yb + 1) * 128],
                                        wrs[gi][0:rloc, :],
                                        start=True, stop=True)
                                fin = pool_up.tile([128, 4, W], f32,
                                                   tag="fin")
                                S.copy(fin[:, 0:gn], psf[:, 0:gn])
                                nc.sync.dma_start(
                                    out=out[yb * 128:(yb + 1) * 128,
                                            nch_base + g0:nch_base + g0 + gn,
                                            :],
                                    in_=fin[:, 0:gn])

                    upsample(17, L1R, L1W, r1t_s, c1_s,
                             lambda chn: ot2[0:L1R, chn, 0:L1W])
                    upsample(34, L2R, L2W, r2t_s, c2_s,
                             lambda chn: l2f[0:L2R, chn, 0:L2W])

    nc.compile()
    return nc


def kernel(x: np.ndarray) -> np.ndarray:
    from concourse.bass_utils import run_bass_kernel_spmd

    assert x.shape == (4, CIN, H, W), x.shape
    x16 = np.ascontiguousarray(x, dtype=np.float32).astype(np.float16)
    B = x.shape[0]

    if "nc" not in _NC_CACHE:
        _NC_CACHE["nc"] = _build_nc()
    nc = _NC_CACHE["nc"]

    consts = [_host_constants(0), _host_constants(1)]
    in_maps = []
    for core in range(N_CORES):
        b, half = core // 2, core % 2
        r1tc, r2tc, c1c, c2c = consts[half]
        r0 = half * HALF
        xmv = x16[b, :, r0:r0 + HALF, :].transpose(1, 0, 2)
        xmv = np.ascontiguousarray(
            xmv.reshape(HALF, CIN, L1W, 4).transpose(0, 1, 3, 2)
            .reshape(HALF, CIN, W))
        if half == 0:
            xhv = x16[b, :, HALF:HALF + HALO, :]
        else:
            xhv = x16[b, :, HALF - HALO:HALF, :]
        xhv = np.ascontiguousarray(
            xhv.reshape(CIN, HALO, L1W, 4).transpose(0, 1, 3, 2)
            .reshape(CIN, HALO, W))
        in_maps.append({
            "xm": xmv, "xh": xhv,
            "r1t": r1tc, "r2t": r2tc, "c1": c1c, "c2": c2c,
        })

    res = run_bass_kernel_spmd(nc, in_maps, core_ids=list(range(N_CORES)))
    outv = np.empty((B, 17 * 3, H, W), np.float32)
    for core in range(N_CORES):
        b, half = core // 2, core % 2
        outv[b, :, half * HALF:(half + 1) * HALF, :] = \
            res.results[core]["out"].transpose(1, 0, 2)
    return outv
